# revision 18
# baseline (speedup 1.0000x reference)
"""Distributed Bass kernel for nn_Attention_12953621365048 (8 TRN2 NeuronCores).

Sharding: 2 batch-groups x 4 head-groups (3 heads/core).
  core c: batch b = c//4, heads 3*(c%4) .. 3*(c%4)+2
Per core: y-KV first then x-QKV (transposed [dim, tok] layout) with
software-pipelined RMSNorm (gpsimd partition reduce) + RoPE, attention with
no-max softmax and a lag-2 scores->exp->AV pipeline, 8-way AllToAll
(block-duplicated across batches), receiver-side batch-select (DVE) so the
projection contracts only 12 head-blocks, projection chains interleaved into
the following head's attention. Host side only shards/gathers.
"""

from contextlib import ExitStack

import numpy as np
import ml_dtypes

import concourse.bass as bass
import concourse.mybir as mybir
import concourse.tile as tile
from concourse import bacc
from concourse import bass_isa
from concourse.bass_utils import run_bass_kernel_spmd

B, N, M, C, H, HD, RD = 2, 2048, 512, 1536, 12, 128, 64
EPS = 1e-6
NHL = 3               # heads per core
S = N + M             # 2560 kv tokens
KT = S // 128         # 20 kv tiles
NQC = N // 512        # 4 q-chunks of 512
CH = 1024             # qkv-phase token chunk
F32 = mybir.dt.float32
AF = mybir.ActivationFunctionType
ALU = mybir.AluOpType
BF16 = mybir.dt.bfloat16
NCT = C // 128        # 12 contraction tiles


def build_nc():
    nc = bacc.Bacc("TRN2", target_bir_lowering=False, debug=False, num_devices=8)

    xT = nc.dram_tensor("xT", [C, N], BF16, kind="ExternalInput").ap()
    yT = nc.dram_tensor("yT", [C, M], BF16, kind="ExternalInput").ap()
    wqkv = nc.dram_tensor("wqkv", [C, 3 * NHL * HD], BF16, kind="ExternalInput").ap()
    wkv = nc.dram_tensor("wkv", [C, 2 * NHL * HD], BF16, kind="ExternalInput").ap()
    wproj = nc.dram_tensor("wproj", [NHL * 4 * 128, C], BF16, kind="ExternalInput").ap()
    wq = nc.dram_tensor("wq", [1, HD], F32, kind="ExternalInput").ap()
    wk = nc.dram_tensor("wk", [1, HD], F32, kind="ExternalInput").ap()
    cs = nc.dram_tensor("cs", [RD, N], BF16, kind="ExternalInput").ap()
    sn = nc.dram_tensor("sn", [RD, N], BF16, kind="ExternalInput").ap()
    ywT = nc.dram_tensor("ywT", [128, M // 128], F32, kind="ExternalInput").ap()
    bpr = nc.dram_tensor("bpr", [1, C], F32, kind="ExternalInput").ap()
    onesb = nc.dram_tensor("onesb", [128, 1], BF16, kind="ExternalInput").ap()
    bsel = nc.dram_tensor("bsel", [128, 1], F32, kind="ExternalInput").ap()
    out = nc.dram_tensor("out", [512, C], F32, kind="ExternalOutput").ap()

    with tile.TileContext(nc) as tc, ExitStack() as ctx:
        # ---------- outer (whole-kernel) pools ----------
        pers = ctx.enter_context(tc.tile_pool(name="persist", bufs=1))
        dram = ctx.enter_context(tc.tile_pool(name="dram", bufs=1, space="DRAM"))
        wpre = ctx.enter_context(tc.tile_pool(name="wpre", bufs=2))
        outp = ctx.enter_context(tc.tile_pool(name="osb", bufs=2))

        onesb_sb = pers.tile([128, 1], BF16, tag="onesb")
        nc.sync.dma_start(onesb_sb[:], onesb)
        eps_sb = pers.tile([1, 1], F32, tag="eps")
        nc.vector.memset(eps_sb[:], EPS)
        wq_sb = pers.tile([128, 1], F32, tag="wq")
        nc.sync.dma_start(wq_sb[:], wq.rearrange("o p -> p o"))
        wk_sb = pers.tile([128, 1], F32, tag="wk")
        nc.sync.dma_start(wk_sb[:], wk.rearrange("o p -> p o"))
        bsel_sb = pers.tile([128, 1], F32, tag="bsel")
        nc.sync.dma_start(bsel_sb[:], bsel)

        # attention bias per kv tile column: 0 for x tokens, log(clip(w)) for y
        bias_sb = pers.tile([128, KT], F32, tag="bias")
        nc.vector.memset(bias_sb[:, 0 : N // 128], 0.0)
        ywT_sb = pers.tile([128, M // 128], F32, tag="ywT")
        nc.sync.dma_start(ywT_sb[:], ywT)
        ywc = pers.tile([128, M // 128], F32, tag="ywc")
        nc.vector.tensor_scalar_max(ywc[:], ywT_sb[:], 1e-4)
        nc.scalar.activation(bias_sb[:, N // 128 : KT], ywc[:], AF.Ln)

        # persistent activations
        qn = [pers.tile([128, N], BF16, tag=f"qn{t}", name=f"qn{t}") for t in range(NHL)]
        kn = [pers.tile([128, S], BF16, tag=f"kn{t}", name=f"kn{t}") for t in range(NHL)]
        v_sb = pers.tile([128, KT * NHL * HD], BF16, tag="v")  # [kv_tile, head, hd]

        a2a_ins = [
            dram.tile([2 * NQC, 128, 512], BF16, name=f"a2ai{t}") for t in range(NHL)
        ]
        a2a_outs = [
            dram.tile([2 * NQC, 128, 512], BF16, name=f"a2ao{t}") for t in range(NHL)
        ]

        def prefetch_w(t):
            wp = wpre.tile([128, 12, 512], BF16, tag="wpre", name=f"wpre{t}")
            for i in range(4):
                nc.sync.dma_start(
                    wp[:, 3 * i : 3 * (i + 1), :],
                    wproj[(t * 4 + i) * 128 : (t * 4 + i + 1) * 128, :],
                )
            return wp

        # ---------- phase A/B: kv (y first), qkv(x), norm, rope ----------
        with ExitStack() as ab:
            csn = ab.enter_context(tc.tile_pool(name="csn", bufs=1))
            wbig = ab.enter_context(tc.tile_pool(name="wbig", bufs=2))
            xtp = ab.enter_context(tc.tile_pool(name="xt", bufs=2))
            sqp = ab.enter_context(tc.tile_pool(name="sq", bufs=2))
            smallp = ab.enter_context(tc.tile_pool(name="small", bufs=3))
            brp = ab.enter_context(tc.tile_pool(name="bcast", bufs=2))
            ropep = ab.enter_context(tc.tile_pool(name="rope", bufs=2))
            psA = ab.enter_context(tc.tile_pool(name="psA", bufs=2, space="PSUM"))
            psV = ab.enter_context(tc.tile_pool(name="psV", bufs=2, space="PSUM"))
            psS = ab.enter_context(tc.tile_pool(name="psS", bufs=1, space="PSUM"))

            def norm_head(raw_ps, dst, w_sb, rope_q0, CHc):
                """RMSNorm over partition dim (HD) + optional RoPE; [128,CHc]."""
                sq = sqp.tile([128, CH], BF16, tag="sq", name="sq")[:, :CHc]
                nc.scalar.activation(sq, raw_ps[:], AF.Square)
                ssq = psS.tile([1, CH], F32, tag="ssq", name="ssq")[:, :CHc]
                for h0 in range(0, CHc, 512):
                    hw = min(512, CHc - h0)
                    nc.tensor.matmul(
                        ssq[:, h0 : h0 + hw],
                        onesb_sb[:],
                        sq[:, h0 : h0 + hw],
                        start=True,
                        stop=True,
                    )
                inv = smallp.tile([1, CH], F32, tag="inv", name="inv")[:, :CHc]
                nc.scalar.activation(
                    inv, ssq, AF.Abs_reciprocal_sqrt, bias=eps_sb[:],
                    scale=1.0 / HD,
                )
                binv = brp.tile([128, CH], F32, tag="binv", name="binv")[:, :CHc]
                nc.gpsimd.partition_broadcast(binv, inv)
                nc.vector.scalar_tensor_tensor(
                    dst, raw_ps[:], w_sb[:], binv, op0=ALU.mult, op1=ALU.mult
                )
                if rope_q0 is not None:
                    hf = RD // 2
                    csc = cs_sb[:, rope_q0 : rope_q0 + CHc]
                    snc = sn_sb[:, rope_q0 : rope_q0 + CHc]
                    sw = ropep.tile([RD, CH], BF16, tag="sw", name="sw")[:, :CHc]
                    nc.scalar.copy(sw[0:hf, :], dst[hf:RD, :])
                    nc.scalar.copy(sw[hf:RD, :], dst[0:hf, :])
                    ma = ropep.tile([RD, CH], BF16, tag="ma", name="ma")[:, :CHc]
                    mb = ropep.tile([RD, CH], BF16, tag="mb", name="mb")[:, :CHc]
                    nc.vector.tensor_mul(ma, dst[0:RD, :], csc)
                    nc.vector.tensor_mul(mb, sw, snc)
                    nc.vector.tensor_add(dst[0:RD, :], ma, mb)

            pend = [None]

            def flush_norm():
                if pend[0] is not None:
                    norm_head(*pend[0])
                    pend[0] = None

            def qkv_chunk(src_sb, w_sb, nqh, q0, kdst_off, vt0, rope, CHc):
                """One CHc-token chunk: q (nqh heads), k (NHL heads), v (NHL heads)."""
                for t in range(nqh + NHL):
                    ps = psA.tile([128, CH], F32, tag="qk", name="qk")[:, :CHc]
                    coff = t * HD
                    for ct in range(NCT):
                        for h0 in range(0, CHc, 512):
                            hw = min(512, CHc - h0)
                            nc.tensor.matmul(
                                ps[:, h0 : h0 + hw],
                                w_sb[:, ct, coff : coff + HD],
                                src_sb[:, ct, h0 : h0 + hw],
                                start=(ct == 0),
                                stop=(ct == NCT - 1),
                            )
                    flush_norm()
                    if t < nqh:
                        pend[0] = (
                            ps, qn[t][:, q0 : q0 + CHc], wq_sb,
                            q0 if rope else None, CHc,
                        )
                    else:
                        pend[0] = (
                            ps,
                            kn[t - nqh][:, kdst_off : kdst_off + CHc],
                            wk_sb,
                            q0 if rope else None,
                            CHc,
                        )
                voff = (nqh + NHL) * HD
                for ts in range(CHc // 128):
                    ps = psV.tile([128, NHL * HD], F32, tag="vps")
                    for ct in range(NCT):
                        nc.tensor.matmul(
                            ps[:],
                            src_sb[:, ct, ts * 128 : (ts + 1) * 128],
                            w_sb[:, ct, voff : voff + NHL * HD],
                            start=(ct == 0),
                            stop=(ct == NCT - 1),
                        )
                    if ts == 0:
                        flush_norm()
                    kvt = vt0 + ts
                    nc.vector.tensor_copy(
                        v_sb[:, kvt * NHL * HD : (kvt + 1) * NHL * HD], ps[:]
                    )

            # --- y-KV first (small DMA working set -> PE starts sooner) ---
            wkv_sb = wbig.tile([128, NCT, 2 * NHL * HD], BF16, tag="wkv", bufs=1)
            yt_sb = xtp.tile([128, NCT, CH], BF16, tag="xt")
            for ct in range(NCT):
                nc.sync.dma_start(
                    wkv_sb[:, ct, : 2 * NHL * HD], wkv[ct * 128 : (ct + 1) * 128, :]
                )
                nc.sync.dma_start(
                    yt_sb[:, ct, :M], yT[ct * 128 : (ct + 1) * 128, :]
                )
            # x/weights stream in behind y
            wqkv_sb = wbig.tile([128, NCT, 3 * NHL * HD], BF16, tag="wqkv", bufs=1)
            xt_first = xtp.tile([128, NCT, CH], BF16, tag="xt", name="xt_first")
            for ct in range(NCT):
                nc.sync.dma_start(
                    wqkv_sb[:, ct, :], wqkv[ct * 128 : (ct + 1) * 128, :]
                )
                nc.sync.dma_start(
                    xt_first[:, ct, :], xT[ct * 128 : (ct + 1) * 128, 0:CH]
                )
            cs_sb = csn.tile([RD, N], BF16, tag="cs")
            nc.sync.dma_start(cs_sb[:], cs)
            sn_sb = csn.tile([RD, N], BF16, tag="sn")
            nc.sync.dma_start(sn_sb[:], sn)
            wp0 = prefetch_w(0)

            qkv_chunk(yt_sb, wkv_sb, 0, 0, N, N // 128, rope=False, CHc=M)
            for qc in range(N // CH):
                q0 = qc * CH
                if qc == 0:
                    xt_sb = xt_first
                else:
                    xt_sb = xtp.tile([128, NCT, CH], BF16, tag="xt")
                    nc.sync.dma_start(
                        xt_sb[:],
                        xT[:, q0 : q0 + CH].rearrange("(ct p) q -> p ct q", p=128),
                    )
                qkv_chunk(xt_sb, wqkv_sb, NHL, q0, q0, q0 // 128, rope=True, CHc=CH)
            flush_norm()

        # ---------- phase C: attention + per-head A2A + interleaved proj ----------
        with ExitStack() as pc:
            expp = pc.enter_context(tc.tile_pool(name="exp", bufs=5))
            exsp = pc.enter_context(tc.tile_pool(name="exs", bufs=3))
            selp = pc.enter_context(tc.tile_pool(name="selp", bufs=2))
            denp = pc.enter_context(tc.tile_pool(name="denp", bufs=2))
            smallc = pc.enter_context(tc.tile_pool(name="smallc", bufs=2))
            accp = pc.enter_context(tc.tile_pool(name="accp", bufs=1))
            pjp = pc.enter_context(tc.tile_pool(name="pjp", bufs=2))
            biasp = pc.enter_context(tc.tile_pool(name="biasp", bufs=1))
            psSc = pc.enter_context(tc.tile_pool(name="psSc", bufs=2, space="PSUM"))
            psAv = pc.enter_context(tc.tile_pool(name="psAv", bufs=2, space="PSUM"))
            psP = pc.enter_context(tc.tile_pool(name="psP", bufs=2, space="PSUM"))

            bpr_sb = biasp.tile([1, C], F32, tag="bpr")
            nc.sync.dma_start(bpr_sb[:], bpr)
            bb_sb = biasp.tile([128, C], F32, tag="bb")
            nc.gpsimd.partition_broadcast(bb_sb[:], bpr_sb[:])

            acc = [
                accp.tile([128, 512], F32, tag=f"acc{i}", name=f"acc{i}")
                for i in range(12)
            ]

            pend_den = [None]  # (exo, av, qc) of the previous chunk
            pend_mul = [None]  # (den128, av, qc) after part 1

            def flush_den1(t):
                if pend_den[0] is None:
                    return
                exo, pav, pqc = pend_den[0]
                pend_den[0] = None
                den128 = denp.tile([128, 512], F32, tag="den128")
                nc.gpsimd.partition_all_reduce(
                    den128[:], exo[:], channels=128,
                    reduce_op=bass_isa.ReduceOp.add,
                )
                pend_mul[0] = (den128, pav, pqc)

            def flush_den2(t):
                if pend_mul[0] is None:
                    return
                den128, pav, pqc = pend_mul[0]
                pend_mul[0] = None
                invd = smallc.tile([128, 512], F32, tag="invd")
                nc.vector.reciprocal_approx_fast(invd[:], den128[:])
                o_sb = outp.tile([128, 512], BF16, tag="o")
                nc.vector.tensor_mul(o_sb[:], pav[:], invd[:])
                nc.sync.dma_start(a2a_ins[t][pqc], o_sb[:])
                nc.sync.dma_start(a2a_ins[t][NQC + pqc], o_sb[:])

            def flush_den(t):
                flush_den1(t)
                flush_den2(t)

            def attention_head(t, hooks):
                for qc in range(NQC):
                    av = psAv.tile([128, 512], F32, tag="av")
                    lvl0, lvl1 = [], []
                    lag = []  # (ex, kp) pending AV issue

                    def issue_av(ex, kp):
                        for kh in range(2):
                            kt = 2 * kp + kh
                            nc.tensor.matmul(
                                av[:],
                                v_sb[
                                    :,
                                    kt * NHL * HD
                                    + t * HD : kt * NHL * HD
                                    + (t + 1) * HD,
                                ],
                                ex[:, kh * 512 : (kh + 1) * 512],
                                start=(kt == 0),
                                stop=(kt == KT - 1),
                            )

                    for kp in range(KT // 2):
                        sc = psSc.tile([128, 1024], F32, tag="sc")
                        for kh in range(2):
                            kt = 2 * kp + kh
                            nc.tensor.matmul(
                                sc[:, kh * 512 : (kh + 1) * 512],
                                kn[t][:, kt * 128 : (kt + 1) * 128],
                                qn[t][:, qc * 512 : (qc + 1) * 512],
                                start=True,
                                stop=True,
                            )
                        if kp == 1:
                            flush_den1(t)
                        if kp == 2:
                            for fn in hooks.get(qc, []):
                                fn()
                        if kp == 5:
                            flush_den2(t)
                        if len(lag) == 2:
                            issue_av(*lag.pop(0))
                        ex = expp.tile([128, 1024], BF16, tag="ex")
                        if kp < 8:
                            nc.scalar.activation(
                                ex[:], sc[:], AF.Exp, bias=bias_sb[:, 0:1]
                            )
                        else:
                            for kh in range(2):
                                kt = 2 * kp + kh
                                nc.scalar.activation(
                                    ex[:, kh * 512 : (kh + 1) * 512],
                                    sc[:, kh * 512 : (kh + 1) * 512],
                                    AF.Exp,
                                    bias=bias_sb[:, kt : kt + 1],
                                )
                        lag.append((ex, kp))
                        exs = exsp.tile([128, 512], BF16, tag="exs", bufs=3)
                        nc.vector.tensor_add(exs[:], ex[:, 0:512], ex[:, 512:1024])
                        lvl0.append(exs)
                        if len(lvl0) == 2:
                            l1 = exsp.tile([128, 512], BF16, tag="l1", bufs=3)
                            nc.vector.tensor_add(l1[:], lvl0[0][:], lvl0[1][:])
                            lvl1.append(l1)
                            lvl0.clear()
                    for e, kp in lag:
                        issue_av(e, kp)
                    t01 = exsp.tile([128, 512], BF16, tag="l2", bufs=3)
                    nc.vector.tensor_add(t01[:], lvl1[0][:], lvl1[1][:])
                    t23 = exsp.tile([128, 512], BF16, tag="l2", bufs=3)
                    nc.vector.tensor_add(t23[:], lvl1[2][:], lvl1[3][:])
                    t03 = exsp.tile([128, 512], BF16, tag="l2", bufs=3)
                    nc.vector.tensor_add(t03[:], t01[:], t23[:])
                    exo = exsp.tile([128, 512], BF16, tag="l2", bufs=3)
                    nc.vector.tensor_add(exo[:], t03[:], lvl1[4][:])
                    pend_den[0] = (exo, av, qc)
                flush_den(t)

            def a2a_head(t):
                nc.gpsimd.collective_compute(
                    "AllToAll",
                    ALU.bypass,
                    replica_groups=[[0, 1, 2, 3, 4, 5, 6, 7]],
                    ins=[a2a_ins[t].opt()],
                    outs=[a2a_outs[t].opt()],
                )

            pjs = {}

            def mk_pjload(t):
                def fn():
                    pj_t = pjp.tile([128, 8, 512], BF16, tag="pj", name=f"pj{t}")
                    nc.sync.dma_start(pj_t[:], a2a_outs[t].rearrange("i p q -> p i q"))
                    pjs[t] = pj_t
                return fn

            def mk_sel(t, i):
                def fn():
                    pj_t = pjs[t]
                    diff = selp.tile([128, 512], BF16, tag="diff")
                    nc.vector.tensor_tensor(
                        diff[:], pj_t[:, i, :], pj_t[:, i + 4, :], ALU.subtract
                    )
                    nc.vector.scalar_tensor_tensor(
                        pj_t[:, i, :], diff[:], bsel_sb[:], pj_t[:, i + 4, :],
                        op0=ALU.mult, op1=ALU.add,
                    )
                return fn

            def mk_chain(t, wp, ch):
                fc, tcc = ch // 4, ch % 4

                def fn():
                    pj_t = pjs[t]
                    pp = psP.tile([128, 512], F32, tag="pp", name=f"pp{t}_{ch}")
                    for i in range(4):
                        nc.tensor.matmul(
                            pp[:],
                            pj_t[:, i, tcc * 128 : (tcc + 1) * 128],
                            wp[:, 3 * i + fc, :],
                            start=(i == 0),
                            stop=(i == 3),
                        )
                    a = acc[ch]
                    if t == 0:
                        nc.vector.tensor_tensor(
                            a[:], bb_sb[:, fc * 512 : (fc + 1) * 512], pp[:], ALU.add
                        )
                    else:
                        nc.vector.tensor_add(a[:], a[:], pp[:])
                    if t == 2:
                        nc.sync.dma_start(
                            out[
                                tcc * 128 : (tcc + 1) * 128,
                                fc * 512 : (fc + 1) * 512,
                            ],
                            a[:],
                        )
                return fn

            wp1 = prefetch_w(1)
            attention_head(0, {})
            attention_head(1, {1: [lambda: a2a_head(0)]})
            wp2 = prefetch_w(2)
            attention_head(2, {0: [mk_pjload(0)], 1: [lambda: a2a_head(1)]})
            a2a_head(2)
            mk_pjload(1)()
            mk_pjload(2)()
            for i in range(4):
                mk_sel(0, i)()
            for i in range(4):
                mk_sel(1, i)()
            for ch in range(12):
                mk_chain(0, wp0, ch)()
            for ch in range(12):
                mk_chain(1, wp1, ch)()
            for i in range(4):
                mk_sel(2, i)()
            for ch in range(12):
                mk_chain(2, wp2, ch)()
    nc.compile()
    return nc


_NC_CACHE = {}


def _get_nc():
    if "nc" not in _NC_CACHE:
        _NC_CACHE["nc"] = build_nc()
    return _NC_CACHE["nc"]


def make_in_maps(x, y, pos, y_token_weights, Wqkv, Wkv, q_norm_w, k_norm_w, Wproj, bproj):
    f = np.float32
    c32 = pos[:, :, 0].T
    s32 = pos[:, :, 1].T
    csT = np.ascontiguousarray(
        np.concatenate([c32, c32], 0).astype(ml_dtypes.bfloat16))   # [64, N]
    snT = np.ascontiguousarray(
        np.concatenate([-s32, s32], 0).astype(ml_dtypes.bfloat16))  # [64, N]
    wqs = (np.asarray(q_norm_w, dtype=f) * np.float32(HD) ** -0.5).reshape(1, HD)
    wkk = np.asarray(k_norm_w, dtype=f).reshape(1, HD)
    Wp = np.asarray(Wproj, dtype=f)
    # wproj rows: [t][i][128] with head h = 3*i + t  (same for all cores)
    wpr = np.zeros((NHL, 4, 128, C), dtype=f)
    for t in range(NHL):
        for i in range(4):
            h = 3 * i + t
            wpr[t, i] = Wp[h * 128 : (h + 1) * 128, :]
    wpr = np.ascontiguousarray(
        wpr.reshape(NHL * 4 * 128, C).astype(ml_dtypes.bfloat16)
    )
    in_maps = []
    for c in range(8):
        b, g = c // 4, c % 4
        heads = [3 * g + i for i in range(NHL)]
        qcols = [Wqkv[:, h * HD : (h + 1) * HD] for h in heads]
        kcols = [Wqkv[:, C + h * HD : C + (h + 1) * HD] for h in heads]
        vcols = [Wqkv[:, 2 * C + h * HD : 2 * C + (h + 1) * HD] for h in heads]
        wqkv_c = np.ascontiguousarray(
            np.concatenate(qcols + kcols + vcols, axis=1), dtype=f
        )
        kcols2 = [Wkv[:, h * HD : (h + 1) * HD] for h in heads]
        vcols2 = [Wkv[:, C + h * HD : C + (h + 1) * HD] for h in heads]
        wkv_c = np.ascontiguousarray(np.concatenate(kcols2 + vcols2, axis=1), dtype=f)
        in_maps.append(
            {
                "xT": np.ascontiguousarray(np.asarray(x)[b].T.astype(ml_dtypes.bfloat16)),
                "yT": np.ascontiguousarray(np.asarray(y)[b].T.astype(ml_dtypes.bfloat16)),
                "wqkv": wqkv_c.astype(ml_dtypes.bfloat16),
                "wkv": wkv_c.astype(ml_dtypes.bfloat16),
                "wproj": wpr,
                "wq": np.ascontiguousarray(wqs),
                "wk": np.ascontiguousarray(wkk),
                "cs": csT,
                "sn": snT,
                "ywT": np.ascontiguousarray(
                    np.asarray(y_token_weights)[b].reshape(M // 128, 128).T, dtype=f
                ),
                "bpr": np.asarray(bproj, dtype=f).reshape(1, C),
                "onesb": np.ones((128, 1), dtype=ml_dtypes.bfloat16),
                "bsel": np.full((128, 1), 1.0 if b == 0 else 0.0, dtype=f),
            }
        )
    return in_maps


def kernel(x, y, pos, y_token_weights, Wqkv, Wkv, q_norm_w, k_norm_w, Wproj, bproj,
           _trace=False):
    x = np.asarray(x, dtype=np.float32)
    y = np.asarray(y, dtype=np.float32)
    pos = np.asarray(pos, dtype=np.float32)
    y_token_weights = np.asarray(y_token_weights, dtype=np.float32)
    nc = _get_nc()
    in_maps = make_in_maps(
        x, y, pos, y_token_weights,
        np.asarray(Wqkv), np.asarray(Wkv), np.asarray(q_norm_w),
        np.asarray(k_norm_w), np.asarray(Wproj), np.asarray(bproj),
    )
    res = run_bass_kernel_spmd(nc, in_maps, core_ids=list(range(8)), trace=_trace)
    outp = np.zeros((B, N, C), dtype=np.float32)
    for c in range(8):
        b, g = c // 4, c % 4
        outp[b, g * 512 : (g + 1) * 512, :] = res.results[c]["out"]
    if _trace:
        return outp, res
    return outp


# revision 19
# speedup vs baseline: 1.1485x; 1.1485x over previous
"""Distributed Bass kernel for nn_Attention_12953621365048 (8 TRN2 NeuronCores).

Sharding: 2 batch-groups x 4 head-groups (3 heads/core).
  core c: batch b = c//4, heads 3*(c%4) .. 3*(c%4)+2
Per core: y-KV first then x-QKV (transposed [dim, tok] layout) with
software-pipelined RMSNorm (gpsimd partition reduce) + RoPE, attention with
no-max softmax and a lag-2 scores->exp->AV pipeline, 8-way AllToAll
(block-duplicated across batches), receiver-side batch-select (DVE) so the
projection contracts only 12 head-blocks, projection chains interleaved into
the following head's attention. Host side only shards/gathers.
"""

from contextlib import ExitStack

import numpy as np
import ml_dtypes

import concourse.bass as bass
import concourse.mybir as mybir
import concourse.tile as tile
from concourse import bacc
from concourse import bass_isa
from concourse.bass_utils import run_bass_kernel_spmd

B, N, M, C, H, HD, RD = 2, 2048, 512, 1536, 12, 128, 64
EPS = 1e-6
NHL = 3               # heads per core
S = N + M             # 2560 kv tokens
KT = S // 128         # 20 kv tiles
NQC = N // 512        # 4 q-chunks of 512
CH = 1024             # qkv-phase token chunk
F32 = mybir.dt.float32
AF = mybir.ActivationFunctionType
ALU = mybir.AluOpType
BF16 = mybir.dt.bfloat16
NCT = C // 128        # 12 contraction tiles


def build_nc():
    nc = bacc.Bacc("TRN2", target_bir_lowering=False, debug=False, num_devices=8)

    xT = nc.dram_tensor("xT", [C, N], BF16, kind="ExternalInput").ap()
    yT = nc.dram_tensor("yT", [C, M], BF16, kind="ExternalInput").ap()
    wqkv = nc.dram_tensor("wqkv", [C, 3 * NHL * HD], BF16, kind="ExternalInput").ap()
    wkv = nc.dram_tensor("wkv", [C, 2 * NHL * HD], BF16, kind="ExternalInput").ap()
    wproj = nc.dram_tensor("wproj", [NHL * 4 * 128, C], BF16, kind="ExternalInput").ap()
    wq = nc.dram_tensor("wq", [1, HD], F32, kind="ExternalInput").ap()
    wk = nc.dram_tensor("wk", [1, HD], F32, kind="ExternalInput").ap()
    cs = nc.dram_tensor("cs", [RD, N], BF16, kind="ExternalInput").ap()
    sn = nc.dram_tensor("sn", [RD, N], BF16, kind="ExternalInput").ap()
    ywT = nc.dram_tensor("ywT", [128, M // 128], F32, kind="ExternalInput").ap()
    bpr = nc.dram_tensor("bpr", [1, C], F32, kind="ExternalInput").ap()
    onesb = nc.dram_tensor("onesb", [128, 1], BF16, kind="ExternalInput").ap()
    bsel = nc.dram_tensor("bsel", [128, 1], F32, kind="ExternalInput").ap()
    out = nc.dram_tensor("out", [512, C], F32, kind="ExternalOutput").ap()

    with tile.TileContext(nc) as tc, ExitStack() as ctx:
        # ---------- outer (whole-kernel) pools ----------
        pers = ctx.enter_context(tc.tile_pool(name="persist", bufs=1))
        dram = ctx.enter_context(tc.tile_pool(name="dram", bufs=1, space="DRAM"))
        wpre = ctx.enter_context(tc.tile_pool(name="wpre", bufs=2))
        outp = ctx.enter_context(tc.tile_pool(name="osb", bufs=2))

        onesb_sb = pers.tile([128, 1], BF16, tag="onesb")
        nc.sync.dma_start(onesb_sb[:], onesb)
        eps_sb = pers.tile([1, 1], F32, tag="eps")
        nc.vector.memset(eps_sb[:], EPS)
        wq_sb = pers.tile([128, 1], F32, tag="wq")
        nc.sync.dma_start(wq_sb[:], wq.rearrange("o p -> p o"))
        wk_sb = pers.tile([128, 1], F32, tag="wk")
        nc.sync.dma_start(wk_sb[:], wk.rearrange("o p -> p o"))
        bsel_sb = pers.tile([128, 1], F32, tag="bsel")
        nc.sync.dma_start(bsel_sb[:], bsel)

        # attention bias per kv tile column: 0 for x tokens, log(clip(w)) for y
        bias_sb = pers.tile([128, KT], F32, tag="bias")
        nc.vector.memset(bias_sb[:, 0 : N // 128], 0.0)
        ywT_sb = pers.tile([128, M // 128], F32, tag="ywT")
        nc.sync.dma_start(ywT_sb[:], ywT)
        ywc = pers.tile([128, M // 128], F32, tag="ywc")
        nc.vector.tensor_scalar_max(ywc[:], ywT_sb[:], 1e-4)
        nc.scalar.activation(bias_sb[:, N // 128 : KT], ywc[:], AF.Ln)

        # persistent activations
        qn = [pers.tile([128, N], BF16, tag=f"qn{t}", name=f"qn{t}") for t in range(NHL)]
        kn = [pers.tile([128, S], BF16, tag=f"kn{t}", name=f"kn{t}") for t in range(NHL)]
        v_sb = pers.tile([128, KT * NHL * HD], BF16, tag="v")  # [kv_tile, head, hd]

        a2a_ins = [
            dram.tile([2 * NQC, 128, 512], BF16, name=f"a2ai{t}") for t in range(NHL)
        ]
        a2a_outs = [
            dram.tile([2 * NQC, 128, 512], BF16, name=f"a2ao{t}") for t in range(NHL)
        ]

        def prefetch_w(t):
            wp = wpre.tile([128, 12, 512], BF16, tag="wpre", name=f"wpre{t}")
            for i in range(4):
                nc.sync.dma_start(
                    wp[:, 3 * i : 3 * (i + 1), :],
                    wproj[(t * 4 + i) * 128 : (t * 4 + i + 1) * 128, :],
                )
            return wp

        # ---------- phase A/B: kv (y first), qkv(x), norm, rope ----------
        with ExitStack() as ab:
            csn = ab.enter_context(tc.tile_pool(name="csn", bufs=1))
            wbig = ab.enter_context(tc.tile_pool(name="wbig", bufs=2))
            xtp = ab.enter_context(tc.tile_pool(name="xt", bufs=2))
            sqp = ab.enter_context(tc.tile_pool(name="sq", bufs=2))
            smallp = ab.enter_context(tc.tile_pool(name="small", bufs=3))
            brp = ab.enter_context(tc.tile_pool(name="bcast", bufs=2))
            ropep = ab.enter_context(tc.tile_pool(name="rope", bufs=2))
            psA = ab.enter_context(tc.tile_pool(name="psA", bufs=2, space="PSUM"))
            psV = ab.enter_context(tc.tile_pool(name="psV", bufs=2, space="PSUM"))
            psS = ab.enter_context(tc.tile_pool(name="psS", bufs=1, space="PSUM"))

            def norm_head(raw_ps, dst, w_sb, rope_q0, CHc):
                """RMSNorm over partition dim (HD) + optional RoPE; [128,CHc]."""
                sq = sqp.tile([128, CH], BF16, tag="sq", name="sq")[:, :CHc]
                nc.scalar.activation(sq, raw_ps[:], AF.Square)
                ssq = psS.tile([1, CH], F32, tag="ssq", name="ssq")[:, :CHc]
                for h0 in range(0, CHc, 512):
                    hw = min(512, CHc - h0)
                    nc.tensor.matmul(
                        ssq[:, h0 : h0 + hw],
                        onesb_sb[:],
                        sq[:, h0 : h0 + hw],
                        start=True,
                        stop=True,
                    )
                inv = smallp.tile([1, CH], F32, tag="inv", name="inv")[:, :CHc]
                nc.scalar.activation(
                    inv, ssq, AF.Abs_reciprocal_sqrt, bias=eps_sb[:],
                    scale=1.0 / HD,
                )
                binv = brp.tile([128, CH], F32, tag="binv", name="binv")[:, :CHc]
                nc.gpsimd.partition_broadcast(binv, inv)
                nc.vector.scalar_tensor_tensor(
                    dst, raw_ps[:], w_sb[:], binv, op0=ALU.mult, op1=ALU.mult
                )
                if rope_q0 is not None:
                    hf = RD // 2
                    csc = cs_sb[:, rope_q0 : rope_q0 + CHc]
                    snc = sn_sb[:, rope_q0 : rope_q0 + CHc]
                    sw = ropep.tile([RD, CH], BF16, tag="sw", name="sw")[:, :CHc]
                    nc.scalar.copy(sw[0:hf, :], dst[hf:RD, :])
                    nc.scalar.copy(sw[hf:RD, :], dst[0:hf, :])
                    ma = ropep.tile([RD, CH], BF16, tag="ma", name="ma")[:, :CHc]
                    mb = ropep.tile([RD, CH], BF16, tag="mb", name="mb")[:, :CHc]
                    nc.vector.tensor_mul(ma, dst[0:RD, :], csc)
                    nc.vector.tensor_mul(mb, sw, snc)
                    nc.vector.tensor_add(dst[0:RD, :], ma, mb)

            pend = [None]

            def flush_norm():
                if pend[0] is not None:
                    norm_head(*pend[0])
                    pend[0] = None

            def qkv_chunk(src_sb, w_sb, nqh, q0, kdst_off, vt0, rope, CHc):
                """One CHc-token chunk: q (nqh heads), k (NHL heads), v (NHL heads)."""
                for t in range(nqh + NHL):
                    ps = psA.tile([128, CH], F32, tag="qk", name="qk")[:, :CHc]
                    coff = t * HD
                    for ct in range(NCT):
                        for h0 in range(0, CHc, 512):
                            hw = min(512, CHc - h0)
                            nc.tensor.matmul(
                                ps[:, h0 : h0 + hw],
                                w_sb[:, ct, coff : coff + HD],
                                src_sb[:, ct, h0 : h0 + hw],
                                start=(ct == 0),
                                stop=(ct == NCT - 1),
                            )
                    flush_norm()
                    if t < nqh:
                        pend[0] = (
                            ps, qn[t][:, q0 : q0 + CHc], wq_sb,
                            q0 if rope else None, CHc,
                        )
                    else:
                        pend[0] = (
                            ps,
                            kn[t - nqh][:, kdst_off : kdst_off + CHc],
                            wk_sb,
                            q0 if rope else None,
                            CHc,
                        )
                voff = (nqh + NHL) * HD
                for ts in range(CHc // 128):
                    ps = psV.tile([128, NHL * HD], F32, tag="vps")
                    for ct in range(NCT):
                        nc.tensor.matmul(
                            ps[:],
                            src_sb[:, ct, ts * 128 : (ts + 1) * 128],
                            w_sb[:, ct, voff : voff + NHL * HD],
                            start=(ct == 0),
                            stop=(ct == NCT - 1),
                        )
                    if ts == 0:
                        flush_norm()
                    kvt = vt0 + ts
                    nc.vector.tensor_copy(
                        v_sb[:, kvt * NHL * HD : (kvt + 1) * NHL * HD], ps[:]
                    )

            # --- y-KV first (small DMA working set -> PE starts sooner) ---
            wkv_sb = wbig.tile([128, NCT, 2 * NHL * HD], BF16, tag="wkv", bufs=1)
            yt_sb = xtp.tile([128, NCT, CH], BF16, tag="xt")
            for ct in range(NCT):
                nc.sync.dma_start(
                    wkv_sb[:, ct, : 2 * NHL * HD], wkv[ct * 128 : (ct + 1) * 128, :]
                )
                nc.sync.dma_start(
                    yt_sb[:, ct, :M], yT[ct * 128 : (ct + 1) * 128, :]
                )
            # x/weights stream in behind y
            wqkv_sb = wbig.tile([128, NCT, 3 * NHL * HD], BF16, tag="wqkv", bufs=1)
            xt_first = xtp.tile([128, NCT, CH], BF16, tag="xt", name="xt_first")
            for ct in range(NCT):
                nc.sync.dma_start(
                    wqkv_sb[:, ct, :], wqkv[ct * 128 : (ct + 1) * 128, :]
                )
                nc.sync.dma_start(
                    xt_first[:, ct, :], xT[ct * 128 : (ct + 1) * 128, 0:CH]
                )
            cs_sb = csn.tile([RD, N], BF16, tag="cs")
            nc.sync.dma_start(cs_sb[:], cs)
            sn_sb = csn.tile([RD, N], BF16, tag="sn")
            nc.sync.dma_start(sn_sb[:], sn)
            wp0 = prefetch_w(0)

            qkv_chunk(yt_sb, wkv_sb, 0, 0, N, N // 128, rope=False, CHc=M)
            for qc in range(N // CH):
                q0 = qc * CH
                if qc == 0:
                    xt_sb = xt_first
                else:
                    xt_sb = xtp.tile([128, NCT, CH], BF16, tag="xt")
                    nc.sync.dma_start(
                        xt_sb[:],
                        xT[:, q0 : q0 + CH].rearrange("(ct p) q -> p ct q", p=128),
                    )
                qkv_chunk(xt_sb, wqkv_sb, NHL, q0, q0, q0 // 128, rope=True, CHc=CH)
            flush_norm()

        # ---------- phase C: attention + per-head A2A + interleaved proj ----------
        with ExitStack() as pc:
            expp = pc.enter_context(tc.tile_pool(name="exp", bufs=5))
            exsp = pc.enter_context(tc.tile_pool(name="exs", bufs=3))
            selp = pc.enter_context(tc.tile_pool(name="selp", bufs=2))
            brp2 = pc.enter_context(tc.tile_pool(name="bcast2", bufs=2))
            smallc = pc.enter_context(tc.tile_pool(name="smallc", bufs=2))
            accp = pc.enter_context(tc.tile_pool(name="accp", bufs=1))
            pjp = pc.enter_context(tc.tile_pool(name="pjp", bufs=2))
            biasp = pc.enter_context(tc.tile_pool(name="biasp", bufs=1))
            psSc = pc.enter_context(tc.tile_pool(name="psSc", bufs=2, space="PSUM"))
            psAv = pc.enter_context(tc.tile_pool(name="psAv", bufs=2, space="PSUM"))
            psDen = pc.enter_context(tc.tile_pool(name="psDen", bufs=1, space="PSUM"))
            psP = pc.enter_context(tc.tile_pool(name="psP", bufs=1, space="PSUM"))

            bpr_sb = biasp.tile([1, C], F32, tag="bpr")
            nc.sync.dma_start(bpr_sb[:], bpr)
            bb_sb = biasp.tile([128, C], F32, tag="bb")
            nc.gpsimd.partition_broadcast(bb_sb[:], bpr_sb[:])

            acc = [
                accp.tile([128, 512], F32, tag=f"acc{i}", name=f"acc{i}")
                for i in range(12)
            ]

            pend_den = [None]  # (exo, av, qc) of the previous chunk
            pend_mul = [None]  # (bden, av, qc) after part 1

            def flush_den1(t):
                if pend_den[0] is None:
                    return
                exo, pav, pqc = pend_den[0]
                pend_den[0] = None
                den = psDen.tile([1, 512], F32, tag="den")
                nc.tensor.matmul(den[:], onesb_sb[:], exo[:], start=True, stop=True)
                invd = smallc.tile([1, 512], F32, tag="invd")
                nc.vector.reciprocal_approx_fast(invd[:], den[:])
                bden = brp2.tile([128, 512], F32, tag="bden")
                nc.gpsimd.partition_broadcast(bden[:], invd[:])
                pend_mul[0] = (bden, pav, pqc)

            def flush_den2(t):
                if pend_mul[0] is None:
                    return
                bden, pav, pqc = pend_mul[0]
                pend_mul[0] = None
                o_sb = outp.tile([128, 512], BF16, tag="o")
                nc.vector.tensor_mul(o_sb[:], pav[:], bden[:])
                nc.sync.dma_start(a2a_ins[t][pqc], o_sb[:])
                nc.sync.dma_start(a2a_ins[t][NQC + pqc], o_sb[:])

            def flush_den(t):
                flush_den1(t)
                flush_den2(t)

            def attention_head(t, hooks):
                for qc in range(NQC):
                    av = psAv.tile([128, 512], F32, tag="av")
                    lvl0, lvl1 = [], []
                    lag = []  # (ex, kp) pending AV issue

                    def issue_av(ex, kp):
                        for kh in range(2):
                            kt = 2 * kp + kh
                            nc.tensor.matmul(
                                av[:],
                                v_sb[
                                    :,
                                    kt * NHL * HD
                                    + t * HD : kt * NHL * HD
                                    + (t + 1) * HD,
                                ],
                                ex[:, kh * 512 : (kh + 1) * 512],
                                start=(kt == 0),
                                stop=(kt == KT - 1),
                            )

                    for kp in range(KT // 2):
                        sc = psSc.tile([128, 1024], F32, tag="sc")
                        for kh in range(2):
                            kt = 2 * kp + kh
                            nc.tensor.matmul(
                                sc[:, kh * 512 : (kh + 1) * 512],
                                kn[t][:, kt * 128 : (kt + 1) * 128],
                                qn[t][:, qc * 512 : (qc + 1) * 512],
                                start=True,
                                stop=True,
                            )
                        if kp == 1:
                            flush_den1(t)
                        if kp == 2:
                            for fn in hooks.get(qc, []):
                                fn()
                        if kp == 5:
                            flush_den2(t)
                        if len(lag) == 2:
                            issue_av(*lag.pop(0))
                        ex = expp.tile([128, 1024], BF16, tag="ex")
                        if kp < 8:
                            nc.scalar.activation(
                                ex[:], sc[:], AF.Exp, bias=bias_sb[:, 0:1]
                            )
                        else:
                            for kh in range(2):
                                kt = 2 * kp + kh
                                nc.scalar.activation(
                                    ex[:, kh * 512 : (kh + 1) * 512],
                                    sc[:, kh * 512 : (kh + 1) * 512],
                                    AF.Exp,
                                    bias=bias_sb[:, kt : kt + 1],
                                )
                        lag.append((ex, kp))
                        exs = exsp.tile([128, 512], BF16, tag="exs", bufs=3)
                        nc.vector.tensor_add(exs[:], ex[:, 0:512], ex[:, 512:1024])
                        lvl0.append(exs)
                        if len(lvl0) == 2:
                            l1 = exsp.tile([128, 512], BF16, tag="l1", bufs=3)
                            nc.vector.tensor_add(l1[:], lvl0[0][:], lvl0[1][:])
                            lvl1.append(l1)
                            lvl0.clear()
                    for e, kp in lag:
                        issue_av(e, kp)
                    t01 = exsp.tile([128, 512], BF16, tag="l2", bufs=3)
                    nc.vector.tensor_add(t01[:], lvl1[0][:], lvl1[1][:])
                    t23 = exsp.tile([128, 512], BF16, tag="l2", bufs=3)
                    nc.vector.tensor_add(t23[:], lvl1[2][:], lvl1[3][:])
                    t03 = exsp.tile([128, 512], BF16, tag="l2", bufs=3)
                    nc.vector.tensor_add(t03[:], t01[:], t23[:])
                    exo = exsp.tile([128, 512], BF16, tag="l2", bufs=3)
                    nc.vector.tensor_add(exo[:], t03[:], lvl1[4][:])
                    pend_den[0] = (exo, av, qc)
                flush_den(t)

            def a2a_head(t):
                nc.gpsimd.collective_compute(
                    "AllToAll",
                    ALU.bypass,
                    replica_groups=[[0, 1, 2, 3, 4, 5, 6, 7]],
                    ins=[a2a_ins[t].opt()],
                    outs=[a2a_outs[t].opt()],
                )

            pjs = {}

            def mk_pjload(t):
                def fn():
                    pj_t = pjp.tile([128, 8, 512], BF16, tag="pj", name=f"pj{t}")
                    nc.sync.dma_start(pj_t[:], a2a_outs[t].rearrange("i p q -> p i q"))
                    pjs[t] = pj_t
                return fn

            def mk_sel(t, i):
                def fn():
                    pj_t = pjs[t]
                    diff = selp.tile([128, 512], BF16, tag="diff")
                    nc.vector.tensor_tensor(
                        diff[:], pj_t[:, i, :], pj_t[:, i + 4, :], ALU.subtract
                    )
                    nc.vector.scalar_tensor_tensor(
                        pj_t[:, i, :], diff[:], bsel_sb[:], pj_t[:, i + 4, :],
                        op0=ALU.mult, op1=ALU.add,
                    )
                return fn

            def mk_chain(t, wp, ch):
                fc, tcc = ch // 4, ch % 4

                def fn():
                    pj_t = pjs[t]
                    pp = psP.tile([128, 512], F32, tag="pp", name=f"pp{t}_{ch}")
                    for i in range(4):
                        nc.tensor.matmul(
                            pp[:],
                            pj_t[:, i, tcc * 128 : (tcc + 1) * 128],
                            wp[:, 3 * i + fc, :],
                            start=(i == 0),
                            stop=(i == 3),
                        )
                    a = acc[ch]
                    if t == 0:
                        nc.vector.tensor_tensor(
                            a[:], bb_sb[:, fc * 512 : (fc + 1) * 512], pp[:], ALU.add
                        )
                    else:
                        nc.vector.tensor_add(a[:], a[:], pp[:])
                    if t == 2:
                        nc.sync.dma_start(
                            out[
                                tcc * 128 : (tcc + 1) * 128,
                                fc * 512 : (fc + 1) * 512,
                            ],
                            a[:],
                        )
                return fn

            wp1 = prefetch_w(1)
            attention_head(0, {})
            a2a_head(0)
            attention_head(1, {})
            a2a_head(1)
            wp2 = prefetch_w(2)
            attention_head(2, {0: [mk_pjload(0)]})
            a2a_head(2)
            mk_pjload(1)()
            mk_pjload(2)()
            for i in range(4):
                mk_sel(0, i)()
            for ch in range(12):
                mk_chain(0, wp0, ch)()
            for i in range(4):
                mk_sel(1, i)()
            for ch in range(12):
                mk_chain(1, wp1, ch)()
            for i in range(4):
                mk_sel(2, i)()
            for ch in range(12):
                mk_chain(2, wp2, ch)()
    nc.compile()
    return nc


_NC_CACHE = {}


def _get_nc():
    if "nc" not in _NC_CACHE:
        _NC_CACHE["nc"] = build_nc()
    return _NC_CACHE["nc"]


def make_in_maps(x, y, pos, y_token_weights, Wqkv, Wkv, q_norm_w, k_norm_w, Wproj, bproj):
    f = np.float32
    c32 = pos[:, :, 0].T
    s32 = pos[:, :, 1].T
    csT = np.ascontiguousarray(
        np.concatenate([c32, c32], 0).astype(ml_dtypes.bfloat16))   # [64, N]
    snT = np.ascontiguousarray(
        np.concatenate([-s32, s32], 0).astype(ml_dtypes.bfloat16))  # [64, N]
    wqs = (np.asarray(q_norm_w, dtype=f) * np.float32(HD) ** -0.5).reshape(1, HD)
    wkk = np.asarray(k_norm_w, dtype=f).reshape(1, HD)
    Wp = np.asarray(Wproj, dtype=f)
    # wproj rows: [t][i][128] with head h = 3*i + t  (same for all cores)
    wpr = np.zeros((NHL, 4, 128, C), dtype=f)
    for t in range(NHL):
        for i in range(4):
            h = 3 * i + t
            wpr[t, i] = Wp[h * 128 : (h + 1) * 128, :]
    wpr = np.ascontiguousarray(
        wpr.reshape(NHL * 4 * 128, C).astype(ml_dtypes.bfloat16)
    )
    in_maps = []
    for c in range(8):
        b, g = c // 4, c % 4
        heads = [3 * g + i for i in range(NHL)]
        qcols = [Wqkv[:, h * HD : (h + 1) * HD] for h in heads]
        kcols = [Wqkv[:, C + h * HD : C + (h + 1) * HD] for h in heads]
        vcols = [Wqkv[:, 2 * C + h * HD : 2 * C + (h + 1) * HD] for h in heads]
        wqkv_c = np.ascontiguousarray(
            np.concatenate(qcols + kcols + vcols, axis=1), dtype=f
        )
        kcols2 = [Wkv[:, h * HD : (h + 1) * HD] for h in heads]
        vcols2 = [Wkv[:, C + h * HD : C + (h + 1) * HD] for h in heads]
        wkv_c = np.ascontiguousarray(np.concatenate(kcols2 + vcols2, axis=1), dtype=f)
        in_maps.append(
            {
                "xT": np.ascontiguousarray(np.asarray(x)[b].T.astype(ml_dtypes.bfloat16)),
                "yT": np.ascontiguousarray(np.asarray(y)[b].T.astype(ml_dtypes.bfloat16)),
                "wqkv": wqkv_c.astype(ml_dtypes.bfloat16),
                "wkv": wkv_c.astype(ml_dtypes.bfloat16),
                "wproj": wpr,
                "wq": np.ascontiguousarray(wqs),
                "wk": np.ascontiguousarray(wkk),
                "cs": csT,
                "sn": snT,
                "ywT": np.ascontiguousarray(
                    np.asarray(y_token_weights)[b].reshape(M // 128, 128).T, dtype=f
                ),
                "bpr": np.asarray(bproj, dtype=f).reshape(1, C),
                "onesb": np.ones((128, 1), dtype=ml_dtypes.bfloat16),
                "bsel": np.full((128, 1), 1.0 if b == 0 else 0.0, dtype=f),
            }
        )
    return in_maps


def kernel(x, y, pos, y_token_weights, Wqkv, Wkv, q_norm_w, k_norm_w, Wproj, bproj,
           _trace=False):
    x = np.asarray(x, dtype=np.float32)
    y = np.asarray(y, dtype=np.float32)
    pos = np.asarray(pos, dtype=np.float32)
    y_token_weights = np.asarray(y_token_weights, dtype=np.float32)
    nc = _get_nc()
    in_maps = make_in_maps(
        x, y, pos, y_token_weights,
        np.asarray(Wqkv), np.asarray(Wkv), np.asarray(q_norm_w),
        np.asarray(k_norm_w), np.asarray(Wproj), np.asarray(bproj),
    )
    res = run_bass_kernel_spmd(nc, in_maps, core_ids=list(range(8)), trace=_trace)
    outp = np.zeros((B, N, C), dtype=np.float32)
    for c in range(8):
        b, g = c // 4, c % 4
        outp[b, g * 512 : (g + 1) * 512, :] = res.results[c]["out"]
    if _trace:
        return outp, res
    return outp


# revision 23
# speedup vs baseline: 1.2359x; 1.0761x over previous
"""Distributed Bass kernel for nn_Attention_12953621365048 (8 TRN2 NeuronCores).

Sharding: 2 batch-groups x 4 head-groups (3 heads/core).
  core c: batch b = c//4, heads 3*(c%4) .. 3*(c%4)+2
Per core: y-KV first then x-QKV (transposed [dim, tok] layout) with
software-pipelined RMSNorm (gpsimd partition reduce) + RoPE, attention with
no-max softmax and a lag-2 scores->exp->AV pipeline, 8-way AllToAll
(block-duplicated across batches), receiver-side batch-select (DVE) so the
projection contracts only 12 head-blocks, projection chains interleaved into
the following head's attention. Host side only shards/gathers.
"""

from contextlib import ExitStack

import numpy as np
import ml_dtypes

import concourse.bass as bass
import concourse.mybir as mybir
import concourse.tile as tile
from concourse import bacc
from concourse import bass_isa
from concourse.bass_utils import run_bass_kernel_spmd

B, N, M, C, H, HD, RD = 2, 2048, 512, 1536, 12, 128, 64
EPS = 1e-6
NHL = 3               # heads per core
S = N + M             # 2560 kv tokens
KT = S // 128         # 20 kv tiles
NQC = N // 512        # 4 q-chunks of 512
CH = 1024             # qkv-phase token chunk
F32 = mybir.dt.float32
AF = mybir.ActivationFunctionType
ALU = mybir.AluOpType
BF16 = mybir.dt.bfloat16
NCT = C // 128        # 12 contraction tiles


def build_nc():
    nc = bacc.Bacc("TRN2", target_bir_lowering=False, debug=False, num_devices=8)

    xT = nc.dram_tensor("xT", [C, N], BF16, kind="ExternalInput").ap()
    yT = nc.dram_tensor("yT", [C, M], BF16, kind="ExternalInput").ap()
    wqkv = nc.dram_tensor("wqkv", [C, 3 * NHL * HD], BF16, kind="ExternalInput").ap()
    wkv = nc.dram_tensor("wkv", [C, 2 * NHL * HD], BF16, kind="ExternalInput").ap()
    wproj = nc.dram_tensor("wproj", [NHL * 4 * 128, C], BF16, kind="ExternalInput").ap()
    wq = nc.dram_tensor("wq", [1, HD], F32, kind="ExternalInput").ap()
    wk = nc.dram_tensor("wk", [1, HD], F32, kind="ExternalInput").ap()
    cs = nc.dram_tensor("cs", [RD, N], BF16, kind="ExternalInput").ap()
    sn = nc.dram_tensor("sn", [RD, N], BF16, kind="ExternalInput").ap()
    ywT = nc.dram_tensor("ywT", [128, M // 128], F32, kind="ExternalInput").ap()
    bpr = nc.dram_tensor("bpr", [1, C], F32, kind="ExternalInput").ap()
    onesb = nc.dram_tensor("onesb", [128, 1], BF16, kind="ExternalInput").ap()
    bsel = nc.dram_tensor("bsel", [128, 1], F32, kind="ExternalInput").ap()
    out = nc.dram_tensor("out", [512, C], F32, kind="ExternalOutput").ap()

    with tile.TileContext(nc) as tc, ExitStack() as ctx:
        # ---------- outer (whole-kernel) pools ----------
        pers = ctx.enter_context(tc.tile_pool(name="persist", bufs=1))
        dram = ctx.enter_context(tc.tile_pool(name="dram", bufs=1, space="DRAM"))
        wpre = ctx.enter_context(tc.tile_pool(name="wpre", bufs=2))
        outp = ctx.enter_context(tc.tile_pool(name="osb", bufs=2))

        onesb_sb = pers.tile([128, 1], BF16, tag="onesb")
        nc.sync.dma_start(onesb_sb[:], onesb)
        eps_sb = pers.tile([1, 1], F32, tag="eps")
        nc.vector.memset(eps_sb[:], EPS)
        wq_sb = pers.tile([128, 1], F32, tag="wq")
        nc.sync.dma_start(wq_sb[:], wq.rearrange("o p -> p o"))
        wk_sb = pers.tile([128, 1], F32, tag="wk")
        nc.sync.dma_start(wk_sb[:], wk.rearrange("o p -> p o"))
        bsel_sb = pers.tile([128, 1], F32, tag="bsel")
        nc.sync.dma_start(bsel_sb[:], bsel)

        # attention bias per kv tile column: 0 for x tokens, log(clip(w)) for y
        bias_sb = pers.tile([128, KT], F32, tag="bias")
        nc.vector.memset(bias_sb[:, 0 : N // 128], 0.0)
        ywT_sb = pers.tile([128, M // 128], F32, tag="ywT")
        nc.sync.dma_start(ywT_sb[:], ywT)
        ywc = pers.tile([128, M // 128], F32, tag="ywc")
        nc.vector.tensor_scalar_max(ywc[:], ywT_sb[:], 1e-4)
        nc.scalar.activation(bias_sb[:, N // 128 : KT], ywc[:], AF.Ln)

        # persistent activations
        qn = [pers.tile([128, N], BF16, tag=f"qn{t}", name=f"qn{t}") for t in range(NHL)]
        kn = [pers.tile([128, S], BF16, tag=f"kn{t}", name=f"kn{t}") for t in range(NHL)]
        v_sb = pers.tile([128, KT * NHL * HD], BF16, tag="v")  # [kv_tile, head, hd]

        a2a_ins = [
            dram.tile([2 * NQC, 128, 512], BF16, name=f"a2ai{t}") for t in range(NHL)
        ]
        a2a_outs = [
            dram.tile([2 * NQC, 128, 512], BF16, name=f"a2ao{t}") for t in range(NHL)
        ]

        def prefetch_w(t):
            wp = wpre.tile([128, 12, 512], BF16, tag="wpre", name=f"wpre{t}")
            for i in range(4):
                nc.sync.dma_start(
                    wp[:, 3 * i : 3 * (i + 1), :],
                    wproj[(t * 4 + i) * 128 : (t * 4 + i + 1) * 128, :],
                )
            return wp

        # ---------- phase A/B: kv (y first), qkv(x), norm, rope ----------
        with ExitStack() as ab:
            csn = ab.enter_context(tc.tile_pool(name="csn", bufs=1))
            wbig = ab.enter_context(tc.tile_pool(name="wbig", bufs=2))
            xtp = ab.enter_context(tc.tile_pool(name="xt", bufs=2))
            sqp = ab.enter_context(tc.tile_pool(name="sq", bufs=2))
            smallp = ab.enter_context(tc.tile_pool(name="small", bufs=3))
            brp = ab.enter_context(tc.tile_pool(name="bcast", bufs=2))
            ropep = ab.enter_context(tc.tile_pool(name="rope", bufs=2))
            psA = ab.enter_context(tc.tile_pool(name="psA", bufs=2, space="PSUM"))
            psV = ab.enter_context(tc.tile_pool(name="psV", bufs=2, space="PSUM"))
            psS = ab.enter_context(tc.tile_pool(name="psS", bufs=1, space="PSUM"))

            def norm_head(raw_ps, dst, w_sb, rope_q0, CHc):
                """RMSNorm over partition dim (HD) + optional RoPE; [128,CHc]."""
                sq = sqp.tile([128, CH], BF16, tag="sq", name="sq")[:, :CHc]
                nc.scalar.activation(sq, raw_ps[:], AF.Square)
                ssq = psS.tile([1, CH], F32, tag="ssq", name="ssq")[:, :CHc]
                for h0 in range(0, CHc, 512):
                    hw = min(512, CHc - h0)
                    nc.tensor.matmul(
                        ssq[:, h0 : h0 + hw],
                        onesb_sb[:],
                        sq[:, h0 : h0 + hw],
                        start=True,
                        stop=True,
                    )
                inv = smallp.tile([1, CH], F32, tag="inv", name="inv")[:, :CHc]
                nc.scalar.activation(
                    inv, ssq, AF.Abs_reciprocal_sqrt, bias=eps_sb[:],
                    scale=1.0 / HD,
                )
                binv = brp.tile([128, CH], F32, tag="binv", name="binv")[:, :CHc]
                nc.gpsimd.partition_broadcast(binv, inv)
                nc.vector.scalar_tensor_tensor(
                    dst, raw_ps[:], w_sb[:], binv, op0=ALU.mult, op1=ALU.mult
                )
                if rope_q0 is not None:
                    hf = RD // 2
                    csc = cs_sb[:, rope_q0 : rope_q0 + CHc]
                    snc = sn_sb[:, rope_q0 : rope_q0 + CHc]
                    sw = ropep.tile([RD, CH], BF16, tag="sw", name="sw")[:, :CHc]
                    nc.scalar.copy(sw[0:hf, :], dst[hf:RD, :])
                    nc.scalar.copy(sw[hf:RD, :], dst[0:hf, :])
                    ma = ropep.tile([RD, CH], BF16, tag="ma", name="ma")[:, :CHc]
                    mb = ropep.tile([RD, CH], BF16, tag="mb", name="mb")[:, :CHc]
                    nc.vector.tensor_mul(ma, dst[0:RD, :], csc)
                    nc.vector.tensor_mul(mb, sw, snc)
                    nc.vector.tensor_add(dst[0:RD, :], ma, mb)

            pend = [None]

            def flush_norm():
                if pend[0] is not None:
                    norm_head(*pend[0])
                    pend[0] = None

            def qkv_chunk(src_sb, w_sb, nqh, q0, kdst_off, vt0, rope, CHc):
                """One CHc-token chunk: q (nqh heads), k (NHL heads), v (NHL heads)."""
                for t in range(nqh + NHL):
                    ps = psA.tile([128, CH], F32, tag="qk", name="qk")[:, :CHc]
                    coff = t * HD
                    for ct in range(NCT):
                        for h0 in range(0, CHc, 512):
                            hw = min(512, CHc - h0)
                            nc.tensor.matmul(
                                ps[:, h0 : h0 + hw],
                                w_sb[:, ct, coff : coff + HD],
                                src_sb[:, ct, h0 : h0 + hw],
                                start=(ct == 0),
                                stop=(ct == NCT - 1),
                            )
                    flush_norm()
                    if t < nqh:
                        pend[0] = (
                            ps, qn[t][:, q0 : q0 + CHc], wq_sb,
                            q0 if rope else None, CHc,
                        )
                    else:
                        pend[0] = (
                            ps,
                            kn[t - nqh][:, kdst_off : kdst_off + CHc],
                            wk_sb,
                            q0 if rope else None,
                            CHc,
                        )
                voff = (nqh + NHL) * HD
                for ts in range(CHc // 128):
                    ps = psV.tile([128, NHL * HD], F32, tag="vps")
                    for ct in range(NCT):
                        nc.tensor.matmul(
                            ps[:],
                            src_sb[:, ct, ts * 128 : (ts + 1) * 128],
                            w_sb[:, ct, voff : voff + NHL * HD],
                            start=(ct == 0),
                            stop=(ct == NCT - 1),
                        )
                    if ts == 0:
                        flush_norm()
                    kvt = vt0 + ts
                    nc.vector.tensor_copy(
                        v_sb[:, kvt * NHL * HD : (kvt + 1) * NHL * HD], ps[:]
                    )

            # --- y-KV first (small DMA working set -> PE starts sooner) ---
            wkv_sb = wbig.tile([128, NCT, 2 * NHL * HD], BF16, tag="wkv", bufs=1)
            yt_sb = xtp.tile([128, NCT, CH], BF16, tag="xt")
            for ct in range(NCT):
                nc.sync.dma_start(
                    wkv_sb[:, ct, : 2 * NHL * HD], wkv[ct * 128 : (ct + 1) * 128, :]
                )
                nc.sync.dma_start(
                    yt_sb[:, ct, :M], yT[ct * 128 : (ct + 1) * 128, :]
                )
            # x/weights stream in behind y
            wqkv_sb = wbig.tile([128, NCT, 3 * NHL * HD], BF16, tag="wqkv", bufs=1)
            xt_first = xtp.tile([128, NCT, CH], BF16, tag="xt", name="xt_first")
            for ct in range(NCT):
                nc.sync.dma_start(
                    wqkv_sb[:, ct, :], wqkv[ct * 128 : (ct + 1) * 128, :]
                )
                nc.sync.dma_start(
                    xt_first[:, ct, :], xT[ct * 128 : (ct + 1) * 128, 0:CH]
                )
            cs_sb = csn.tile([RD, N], BF16, tag="cs")
            nc.sync.dma_start(cs_sb[:], cs)
            sn_sb = csn.tile([RD, N], BF16, tag="sn")
            nc.sync.dma_start(sn_sb[:], sn)
            wp0 = prefetch_w(0)

            qkv_chunk(yt_sb, wkv_sb, 0, 0, N, N // 128, rope=False, CHc=M)
            for qc in range(N // CH):
                q0 = qc * CH
                if qc == 0:
                    xt_sb = xt_first
                else:
                    xt_sb = xtp.tile([128, NCT, CH], BF16, tag="xt")
                    nc.sync.dma_start(
                        xt_sb[:],
                        xT[:, q0 : q0 + CH].rearrange("(ct p) q -> p ct q", p=128),
                    )
                qkv_chunk(xt_sb, wqkv_sb, NHL, q0, q0, q0 // 128, rope=True, CHc=CH)
            flush_norm()

        # ---------- phase C: attention + per-head A2A + interleaved proj ----------
        with ExitStack() as pc:
            expp = pc.enter_context(tc.tile_pool(name="exp", bufs=5))
            exsp = pc.enter_context(tc.tile_pool(name="exs", bufs=3))
            selp = pc.enter_context(tc.tile_pool(name="selp", bufs=2))
            brp2 = pc.enter_context(tc.tile_pool(name="bcast2", bufs=2))
            smallc = pc.enter_context(tc.tile_pool(name="smallc", bufs=2))
            accp = pc.enter_context(tc.tile_pool(name="accp", bufs=1))
            pjp = pc.enter_context(tc.tile_pool(name="pjp", bufs=2))
            biasp = pc.enter_context(tc.tile_pool(name="biasp", bufs=1))
            psSc = pc.enter_context(tc.tile_pool(name="psSc", bufs=2, space="PSUM"))
            psAv = pc.enter_context(tc.tile_pool(name="psAv", bufs=2, space="PSUM"))
            psDen = pc.enter_context(tc.tile_pool(name="psDen", bufs=1, space="PSUM"))
            psP = pc.enter_context(tc.tile_pool(name="psP", bufs=1, space="PSUM"))

            bpr_sb = biasp.tile([1, C], F32, tag="bpr")
            nc.sync.dma_start(bpr_sb[:], bpr)
            bb_sb = biasp.tile([128, C], F32, tag="bb")
            nc.gpsimd.partition_broadcast(bb_sb[:], bpr_sb[:])

            acc = [
                accp.tile([128, 512], F32, tag=f"acc{i}", name=f"acc{i}")
                for i in range(12)
            ]

            pend_den = [None]  # (exo, av, qc) of the previous chunk
            pend_mul = [None]  # (bden, av, qc) after part 1

            def flush_den1(t):
                if pend_den[0] is None:
                    return
                exo, pav, pqc = pend_den[0]
                pend_den[0] = None
                den = psDen.tile([1, 512], F32, tag="den")
                nc.tensor.matmul(den[:], onesb_sb[:], exo[:], start=True, stop=True)
                invd = smallc.tile([1, 512], F32, tag="invd")
                nc.vector.reciprocal_approx_fast(invd[:], den[:])
                bden = brp2.tile([128, 512], F32, tag="bden")
                nc.gpsimd.partition_broadcast(bden[:], invd[:])
                pend_mul[0] = (bden, pav, pqc)

            def flush_den2(t):
                if pend_mul[0] is None:
                    return
                bden, pav, pqc = pend_mul[0]
                pend_mul[0] = None
                o_sb = outp.tile([128, 512], BF16, tag="o")
                nc.vector.tensor_mul(o_sb[:], pav[:], bden[:])
                nc.sync.dma_start(a2a_ins[t][pqc], o_sb[:])
                nc.sync.dma_start(a2a_ins[t][NQC + pqc], o_sb[:])

            def flush_den(t):
                flush_den1(t)
                flush_den2(t)

            def attention_head(t, hooks):
                for qc in range(NQC):
                    av = psAv.tile([128, 512], F32, tag="av")
                    lvl0, lvl1 = [], []
                    lag = []  # (ex, kp) pending AV issue

                    def issue_av(ex, kp):
                        for kh in range(2):
                            kt = 2 * kp + kh
                            nc.tensor.matmul(
                                av[:],
                                v_sb[
                                    :,
                                    kt * NHL * HD
                                    + t * HD : kt * NHL * HD
                                    + (t + 1) * HD,
                                ],
                                ex[:, kh * 512 : (kh + 1) * 512],
                                start=(kt == 0),
                                stop=(kt == KT - 1),
                            )

                    for kp in range(KT // 2):
                        sc = psSc.tile([128, 1024], F32, tag="sc")
                        for kh in range(2):
                            kt = 2 * kp + kh
                            nc.tensor.matmul(
                                sc[:, kh * 512 : (kh + 1) * 512],
                                kn[t][:, kt * 128 : (kt + 1) * 128],
                                qn[t][:, qc * 512 : (qc + 1) * 512],
                                start=True,
                                stop=True,
                            )
                        if kp == 1:
                            flush_den1(t)
                        if kp == 2:
                            for fn in hooks.get(qc, []):
                                fn()
                        if kp == 5:
                            flush_den2(t)
                        if len(lag) == 2:
                            issue_av(*lag.pop(0))
                        ex = expp.tile([128, 1024], BF16, tag="ex")
                        if kp < 8:
                            nc.scalar.activation(
                                ex[:], sc[:], AF.Exp, bias=bias_sb[:, 0:1]
                            )
                        else:
                            for kh in range(2):
                                kt = 2 * kp + kh
                                nc.scalar.activation(
                                    ex[:, kh * 512 : (kh + 1) * 512],
                                    sc[:, kh * 512 : (kh + 1) * 512],
                                    AF.Exp,
                                    bias=bias_sb[:, kt : kt + 1],
                                )
                        lag.append((ex, kp))
                        exs = exsp.tile([128, 512], BF16, tag="exs", bufs=3)
                        nc.vector.tensor_add(exs[:], ex[:, 0:512], ex[:, 512:1024])
                        lvl0.append(exs)
                        if len(lvl0) == 2:
                            l1 = exsp.tile([128, 512], BF16, tag="l1", bufs=3)
                            nc.vector.tensor_add(l1[:], lvl0[0][:], lvl0[1][:])
                            lvl1.append(l1)
                            lvl0.clear()
                    for e, kp in lag:
                        issue_av(e, kp)
                    t01 = exsp.tile([128, 512], BF16, tag="l2", bufs=3)
                    nc.vector.tensor_add(t01[:], lvl1[0][:], lvl1[1][:])
                    t23 = exsp.tile([128, 512], BF16, tag="l2", bufs=3)
                    nc.vector.tensor_add(t23[:], lvl1[2][:], lvl1[3][:])
                    t03 = exsp.tile([128, 512], BF16, tag="l2", bufs=3)
                    nc.vector.tensor_add(t03[:], t01[:], t23[:])
                    exo = exsp.tile([128, 512], BF16, tag="l2", bufs=3)
                    nc.vector.tensor_add(exo[:], t03[:], lvl1[4][:])
                    pend_den[0] = (exo, av, qc)
                flush_den(t)

            def a2a_head(t):
                nc.gpsimd.collective_compute(
                    "AllToAll",
                    ALU.bypass,
                    replica_groups=[[0, 1, 2, 3, 4, 5, 6, 7]],
                    ins=[a2a_ins[t].opt()],
                    outs=[a2a_outs[t].opt()],
                )

            pjs = {}

            def mk_pjload(t):
                def fn():
                    pj_t = pjp.tile([128, 8, 512], BF16, tag="pj", name=f"pj{t}")
                    nc.sync.dma_start(pj_t[:], a2a_outs[t].rearrange("i p q -> p i q"))
                    pjs[t] = pj_t
                return fn

            def mk_sel(t, i):
                def fn():
                    pj_t = pjs[t]
                    diff = selp.tile([128, 512], BF16, tag="diff")
                    nc.vector.tensor_tensor(
                        diff[:], pj_t[:, i, :], pj_t[:, i + 4, :], ALU.subtract
                    )
                    nc.vector.scalar_tensor_tensor(
                        pj_t[:, i, :], diff[:], bsel_sb[:], pj_t[:, i + 4, :],
                        op0=ALU.mult, op1=ALU.add,
                    )
                return fn

            def mk_chain(t, wp, ch):
                fc, tcc = ch // 4, ch % 4

                def fn():
                    pj_t = pjs[t]
                    pool, ptag = (psP, "pp") if ch % 2 == 0 else (psAv, "av")
                    pp = pool.tile([128, 512], F32, tag=ptag, name=f"pp{t}_{ch}")
                    for i in range(4):
                        nc.tensor.matmul(
                            pp[:],
                            pj_t[:, i, tcc * 128 : (tcc + 1) * 128],
                            wp[:, 3 * i + fc, :],
                            start=(i == 0),
                            stop=(i == 3),
                        )
                    a = acc[ch]
                    if t == 0:
                        nc.vector.tensor_tensor(
                            a[:], bb_sb[:, fc * 512 : (fc + 1) * 512], pp[:], ALU.add
                        )
                    else:
                        nc.vector.tensor_add(a[:], a[:], pp[:])
                    if t == 2:
                        nc.sync.dma_start(
                            out[
                                tcc * 128 : (tcc + 1) * 128,
                                fc * 512 : (fc + 1) * 512,
                            ],
                            a[:],
                        )
                return fn

            wp1 = prefetch_w(1)
            attention_head(0, {})
            a2a_head(0)
            attention_head(1, {})
            a2a_head(1)
            attention_head(2, {0: [mk_pjload(0)]})
            a2a_head(2)
            mk_pjload(1)()
            mk_pjload(2)()
            wp2 = prefetch_w(2)
            for i in range(4):
                mk_sel(0, i)()
            for i in range(4):
                mk_sel(1, i)()
            for ch in range(12):
                mk_chain(0, wp0, ch)()
            for ch in range(12):
                mk_chain(1, wp1, ch)()
            for i in range(4):
                mk_sel(2, i)()
            for ch in range(12):
                mk_chain(2, wp2, ch)()
    nc.compile()
    return nc


_NC_CACHE = {}


def _get_nc():
    if "nc" not in _NC_CACHE:
        _NC_CACHE["nc"] = build_nc()
    return _NC_CACHE["nc"]


def make_in_maps(x, y, pos, y_token_weights, Wqkv, Wkv, q_norm_w, k_norm_w, Wproj, bproj):
    f = np.float32
    c32 = pos[:, :, 0].T
    s32 = pos[:, :, 1].T
    csT = np.ascontiguousarray(
        np.concatenate([c32, c32], 0).astype(ml_dtypes.bfloat16))   # [64, N]
    snT = np.ascontiguousarray(
        np.concatenate([-s32, s32], 0).astype(ml_dtypes.bfloat16))  # [64, N]
    wqs = (np.asarray(q_norm_w, dtype=f) * np.float32(HD) ** -0.5).reshape(1, HD)
    wkk = np.asarray(k_norm_w, dtype=f).reshape(1, HD)
    Wp = np.asarray(Wproj, dtype=f)
    # wproj rows: [t][i][128] with head h = 3*i + t  (same for all cores)
    wpr = np.zeros((NHL, 4, 128, C), dtype=f)
    for t in range(NHL):
        for i in range(4):
            h = 3 * i + t
            wpr[t, i] = Wp[h * 128 : (h + 1) * 128, :]
    wpr = np.ascontiguousarray(
        wpr.reshape(NHL * 4 * 128, C).astype(ml_dtypes.bfloat16)
    )
    in_maps = []
    for c in range(8):
        b, g = c // 4, c % 4
        heads = [3 * g + i for i in range(NHL)]
        qcols = [Wqkv[:, h * HD : (h + 1) * HD] for h in heads]
        kcols = [Wqkv[:, C + h * HD : C + (h + 1) * HD] for h in heads]
        vcols = [Wqkv[:, 2 * C + h * HD : 2 * C + (h + 1) * HD] for h in heads]
        wqkv_c = np.ascontiguousarray(
            np.concatenate(qcols + kcols + vcols, axis=1), dtype=f
        )
        kcols2 = [Wkv[:, h * HD : (h + 1) * HD] for h in heads]
        vcols2 = [Wkv[:, C + h * HD : C + (h + 1) * HD] for h in heads]
        wkv_c = np.ascontiguousarray(np.concatenate(kcols2 + vcols2, axis=1), dtype=f)
        in_maps.append(
            {
                "xT": np.ascontiguousarray(np.asarray(x)[b].T.astype(ml_dtypes.bfloat16)),
                "yT": np.ascontiguousarray(np.asarray(y)[b].T.astype(ml_dtypes.bfloat16)),
                "wqkv": wqkv_c.astype(ml_dtypes.bfloat16),
                "wkv": wkv_c.astype(ml_dtypes.bfloat16),
                "wproj": wpr,
                "wq": np.ascontiguousarray(wqs),
                "wk": np.ascontiguousarray(wkk),
                "cs": csT,
                "sn": snT,
                "ywT": np.ascontiguousarray(
                    np.asarray(y_token_weights)[b].reshape(M // 128, 128).T, dtype=f
                ),
                "bpr": np.asarray(bproj, dtype=f).reshape(1, C),
                "onesb": np.ones((128, 1), dtype=ml_dtypes.bfloat16),
                "bsel": np.full((128, 1), 1.0 if b == 0 else 0.0, dtype=f),
            }
        )
    return in_maps


def kernel(x, y, pos, y_token_weights, Wqkv, Wkv, q_norm_w, k_norm_w, Wproj, bproj,
           _trace=False):
    x = np.asarray(x, dtype=np.float32)
    y = np.asarray(y, dtype=np.float32)
    pos = np.asarray(pos, dtype=np.float32)
    y_token_weights = np.asarray(y_token_weights, dtype=np.float32)
    nc = _get_nc()
    in_maps = make_in_maps(
        x, y, pos, y_token_weights,
        np.asarray(Wqkv), np.asarray(Wkv), np.asarray(q_norm_w),
        np.asarray(k_norm_w), np.asarray(Wproj), np.asarray(bproj),
    )
    res = run_bass_kernel_spmd(nc, in_maps, core_ids=list(range(8)), trace=_trace)
    outp = np.zeros((B, N, C), dtype=np.float32)
    for c in range(8):
        b, g = c // 4, c % 4
        outp[b, g * 512 : (g + 1) * 512, :] = res.results[c]["out"]
    if _trace:
        return outp, res
    return outp
